# revision 2
# baseline (speedup 1.0000x reference)
"""MaxPool3d (kernel=3, stride=2, padding=1) on Trainium2, 8 NeuronCores.

Input  x: (2, 32, 128, 128, 128) f32  ->  Output: (2, 32, 64, 64, 64) f32.

Sharding: the 64 (b, c) slices are data-parallel; each of the 8 cores gets 8
slices, processed as 4 slice-pairs (a pair packs 2 slices into the 128 SBUF
partitions: partition 64*s + d//2 holds depth rows 2k/2k+1 of slice s in the
free-dim parity slot).

Per-core algorithm (separable max pooling H -> W -> D), fp16 intermediates:
  - Load 33 h-rows (one-row overlap between chunks, h0-1..h0+31) of both
    depth parities as f32; ScalarE casts to fp16.  fp16 halves DVE time for
    every unit-stride max (2x_1P perf mode) and max() commutes with the
    monotone f32->fp16 rounding, so the result equals fp16(true max):
    rel err <= 2^-11.
  - H pool on the free h axis at 2x: F = max(r[2j], r[2j+1], r[2j+2]).
  - W pool on the free w axis (stride-2 sources, 1x): G = max over w window.
  - D pool across partitions: out = max(G_E, G_O, G_O shifted down one
    partition); the shift is a small SBUF->SBUF DMA; rows 0/64 are covered
    by an idempotent duplicate copy.
  - ScalarE casts the fp16 result to f32, HWDGE stores it.

DMA: all big input loads ride the SP (nc.sync) HWDGE ring back-to-back to
keep HBM saturated; casts, shifts, stores and the tiny first-chunk pad rows
ride the ACT (nc.scalar) ring.
"""

import os
import sys

sys.path.insert(0, "/opt/trn_rl_repo")

import numpy as np

# Shapes (hardcoded per problem spec)
B, C, D, H, W = 2, 32, 128, 128, 128
OD, OH, OW = 64, 64, 64
N_CORES = 8
SLICES_PER_CORE = (B * C) // N_CORES  # 8
PAIRS = SLICES_PER_CORE // 2  # 4
HC = 32  # h rows pooled per chunk (16 output rows); tile holds HC+1 rows
NCH = H // HC  # 4

_cache = {}


def _build():
    import concourse.mybir as mybir
    from concourse import bacc
    from concourse.tile import TileContext

    f32 = mybir.dt.float32
    f16 = mybir.dt.float16
    nc = bacc.Bacc()
    x_ext = nc.declare_dram_parameter(
        "x_shard", [SLICES_PER_CORE, D, H, W], f32, isOutput=False
    )
    y_ext = nc.declare_dram_parameter(
        "y_shard", [SLICES_PER_CORE, OD, OH, OW], f32, isOutput=True
    )

    with TileContext(nc) as tc:
        with (
            tc.tile_pool(name="xpool", bufs=3) as xpool,
            tc.tile_pool(name="hpool", bufs=2) as hpool,
            tc.tile_pool(name="fpool", bufs=2) as fpool,
            tc.tile_pool(name="gpool", bufs=2) as gpool,
            tc.tile_pool(name="ypool", bufs=2) as ypool,
        ):
            for p in range(PAIRS):
                s0 = 2 * p
                for c in range(NCH):
                    h0 = HC * c
                    oh0 = h0 // 2
                    ohc = HC // 2  # 16 output rows per chunk
                    # ---- load 33 h rows of both parities (f32) ----
                    xt = xpool.tile([128, 2, HC + 1, W], f32, name="xt", tag="xt")
                    for par in (0, 1):
                        if c == 0:
                            nc.sync.dma_start(
                                out=xt[:, par : par + 1, 1 : HC + 1, :],
                                in_=x_ext[s0 : s0 + 2, par : D : 2, 0:HC, :],
                            )
                            # h = -1 pad row: duplicate row 0 (max-idempotent)
                            nc.scalar.dma_start(
                                out=xt[:, par : par + 1, 0:1, :],
                                in_=x_ext[s0 : s0 + 2, par : D : 2, 0:1, :],
                            )
                        else:
                            nc.sync.dma_start(
                                out=xt[:, par : par + 1, 0 : HC + 1, :],
                                in_=x_ext[s0 : s0 + 2, par : D : 2, h0 - 1 : h0 + HC, :],
                            )
                    # ---- cast to fp16 (ScalarE) ----
                    xh = hpool.tile([128, 2, HC + 1, W], f16, name="xh", tag="xh")
                    nc.scalar.copy(out=xh, in_=xt)
                    # ---- H pool (free axis, fp16 2x): 33 rows -> 16 ----
                    Ft = fpool.tile([128, 2, ohc, W], f16, name="Ft", tag="Ft")
                    nc.vector.tensor_max(
                        out=Ft,
                        in0=xh[:, :, 0:HC:2, :],
                        in1=xh[:, :, 1:HC:2, :],
                    )
                    nc.vector.tensor_max(
                        out=Ft,
                        in0=Ft,
                        in1=xh[:, :, 2 : HC + 1 : 2, :],
                    )
                    # ---- W pool (free axis, stride-2 sources, 1x) ----
                    Gt = gpool.tile([128, 2, ohc, OW], f16, name="Gt", tag="Gt")
                    nc.vector.tensor_max(
                        out=Gt,
                        in0=Ft[:, :, :, 0:W:2],
                        in1=Ft[:, :, :, 1:W:2],
                    )
                    nc.vector.tensor_max(
                        out=Gt[:, :, :, 1:OW],
                        in0=Gt[:, :, :, 1:OW],
                        in1=Ft[:, :, :, 1 : W - 2 : 2],
                    )
                    # ---- D pool (partition axis) ----
                    # shifted copy of the odd slab: Gs[k] = G_O[k-1]; rows 0
                    # and 64 get the idempotent unshifted value.
                    Gs = gpool.tile([128, 1, ohc, OW], f16, name="Gs", tag="Gs")
                    nc.scalar.dma_start(
                        out=Gs[1:64], in_=Gt[0:63, 1:2, :, :]
                    )
                    nc.scalar.dma_start(
                        out=Gs[65:128], in_=Gt[64:127, 1:2, :, :]
                    )
                    nc.scalar.dma_start(
                        out=Gs[0:65:64], in_=Gt[0:65:64, 1:2, :, :]
                    )
                    Yh = ypool.tile([128, 1, ohc, OW], f16, name="Yh", tag="Yh")
                    nc.vector.tensor_max(
                        out=Yh, in0=Gt[:, 0:1, :, :], in1=Gt[:, 1:2, :, :]
                    )
                    nc.vector.tensor_max(out=Yh, in0=Yh, in1=Gs)
                    # ---- cast to f32 (ScalarE) and store ----
                    Yf = ypool.tile([128, 1, ohc, OW], f32, name="Yf", tag="Yf")
                    nc.scalar.copy(out=Yf, in_=Yh)
                    nc.scalar.dma_start(
                        out=y_ext[s0 : s0 + 2, :, oh0 : oh0 + ohc, :], in_=Yf
                    )
    nc.compile()
    return nc


def _get_nc():
    if "nc" not in _cache:
        _cache["nc"] = _build()
    return _cache["nc"]


def run(x: np.ndarray, **spmd_kwargs):
    """Run the SPMD kernel; returns the BassKernelResults (for tracing)."""
    from concourse.bass_utils import run_bass_kernel_spmd

    nc = _get_nc()
    xs = np.ascontiguousarray(x, dtype=np.float32).reshape(B * C, D, H, W)
    in_maps = [
        {"x_shard": np.ascontiguousarray(xs[SLICES_PER_CORE * i : SLICES_PER_CORE * (i + 1)])}
        for i in range(N_CORES)
    ]
    return run_bass_kernel_spmd(nc, in_maps, list(range(N_CORES)), **spmd_kwargs)


def kernel(x: np.ndarray) -> np.ndarray:
    res = run(x)
    out = np.stack([res.results[i]["y_shard"] for i in range(N_CORES)])
    return out.reshape(B, C, OD, OH, OW)


# revision 3
# speedup vs baseline: 1.3404x; 1.3404x over previous
"""MaxPool3d (kernel=3, stride=2, padding=1) on Trainium2, 8 NeuronCores.

Input  x: (2, 32, 128, 128, 128) f32  ->  Output: (2, 32, 64, 64, 64) f32.

Sharding: the 64 (b, c) slices are data-parallel; each of the 8 cores gets 8
slices, processed as 4 slice-pairs (a pair packs 2 slices into the 128 SBUF
partitions: partition 64*s + d//2 holds depth rows 2k/2k+1 of slice s in the
free-dim parity slot).

Per-core algorithm (separable max pooling H -> W -> D), fp16 intermediates:
  - SWDGE (gpsimd) cast-loads stream x f32 from HBM into fp16 SBUF tiles
    (33 h-rows per chunk, one-row overlap).  fp16 gives the DVE 2x_1P mode
    on every unit-stride max, and max() commutes with the monotone f32->fp16
    rounding, so the result equals fp16(true max): rel err <= 2^-11.
  - H pool on the free h axis at 2x: F = max(r[2j], r[2j+1], r[2j+2]).
  - W pool on the free w axis (stride-2 sources, 1x): G = max over w window.
  - D pool across partitions: out = max(G_E, G_O, G_O shifted down one
    partition); the shift is a small SBUF->SBUF DMA; rows 0/64 are covered
    by an idempotent duplicate copy.
  - ScalarE casts the fp16 result to f32, HWDGE stores it.
"""

import os
import sys

sys.path.insert(0, "/opt/trn_rl_repo")

import numpy as np

# Shapes (hardcoded per problem spec)
B, C, D, H, W = 2, 32, 128, 128, 128
OD, OH, OW = 64, 64, 64
N_CORES = 8
SLICES_PER_CORE = (B * C) // N_CORES  # 8
PAIRS = SLICES_PER_CORE // 2  # 4
HC = 32  # h rows pooled per chunk (16 output rows); tile holds HC+1 rows
NCH = H // HC  # 4

_cache = {}


def _build():
    import concourse.mybir as mybir
    from concourse import bacc
    from concourse.tile import TileContext

    f32 = mybir.dt.float32
    f16 = mybir.dt.float16
    nc = bacc.Bacc()
    x_ext = nc.declare_dram_parameter(
        "x_shard", [SLICES_PER_CORE, D, H, W], f32, isOutput=False
    )
    y_ext = nc.declare_dram_parameter(
        "y_shard", [SLICES_PER_CORE, OD, OH, OW], f32, isOutput=True
    )

    with TileContext(nc) as tc:
        with (
            tc.tile_pool(name="hpool", bufs=3) as hpool,
            tc.tile_pool(name="fpool", bufs=2) as fpool,
            tc.tile_pool(name="gpool", bufs=2) as gpool,
            tc.tile_pool(name="ypool", bufs=2) as ypool,
        ):
            for p in range(PAIRS):
                s0 = 2 * p
                for c in range(NCH):
                    h0 = HC * c
                    oh0 = h0 // 2
                    ohc = HC // 2  # 16 output rows per chunk
                    # ---- SWDGE cast-load: 33 h rows, both parities ----
                    xh = hpool.tile([128, 2, HC + 1, W], f16, name="xh", tag="xh")
                    for par in (0, 1):
                        if c == 0:
                            nc.gpsimd.dma_start(
                                out=xh[:, par : par + 1, 1 : HC + 1, :],
                                in_=x_ext[s0 : s0 + 2, par : D : 2, 0:HC, :],
                            )
                            # h = -1 pad row: duplicate row 0 (max-idempotent)
                            nc.gpsimd.dma_start(
                                out=xh[:, par : par + 1, 0:1, :],
                                in_=x_ext[s0 : s0 + 2, par : D : 2, 0:1, :],
                            )
                        else:
                            nc.gpsimd.dma_start(
                                out=xh[:, par : par + 1, 0 : HC + 1, :],
                                in_=x_ext[s0 : s0 + 2, par : D : 2, h0 - 1 : h0 + HC, :],
                            )
                    # ---- H pool (free axis, fp16 2x): 33 rows -> 16 ----
                    Ft = fpool.tile([128, 2, ohc, W], f16, name="Ft", tag="Ft")
                    nc.vector.tensor_max(
                        out=Ft,
                        in0=xh[:, :, 0:HC:2, :],
                        in1=xh[:, :, 1:HC:2, :],
                    )
                    nc.vector.tensor_max(
                        out=Ft,
                        in0=Ft,
                        in1=xh[:, :, 2 : HC + 1 : 2, :],
                    )
                    # ---- W pool (free axis, stride-2 sources, 1x) ----
                    Gt = gpool.tile([128, 2, ohc, OW], f16, name="Gt", tag="Gt")
                    nc.vector.tensor_max(
                        out=Gt,
                        in0=Ft[:, :, :, 0:W:2],
                        in1=Ft[:, :, :, 1:W:2],
                    )
                    nc.vector.tensor_max(
                        out=Gt[:, :, :, 1:OW],
                        in0=Gt[:, :, :, 1:OW],
                        in1=Ft[:, :, :, 1 : W - 2 : 2],
                    )
                    # ---- D pool (partition axis) ----
                    # shifted copy of the odd slab: Gs[k] = G_O[k-1]; rows 0
                    # and 64 get the idempotent unshifted value.
                    Gs = gpool.tile([128, 1, ohc, OW], f16, name="Gs", tag="Gs")
                    nc.sync.dma_start(
                        out=Gs[1:64], in_=Gt[0:63, 1:2, :, :]
                    )
                    nc.sync.dma_start(
                        out=Gs[65:128], in_=Gt[64:127, 1:2, :, :]
                    )
                    nc.sync.dma_start(
                        out=Gs[0:65:64], in_=Gt[0:65:64, 1:2, :, :]
                    )
                    Yh = ypool.tile([128, 1, ohc, OW], f16, name="Yh", tag="Yh")
                    nc.vector.tensor_max(
                        out=Yh, in0=Gt[:, 0:1, :, :], in1=Gt[:, 1:2, :, :]
                    )
                    nc.vector.tensor_max(out=Yh, in0=Yh, in1=Gs)
                    # ---- cast to f32 (ScalarE) and store ----
                    Yf = ypool.tile([128, 1, ohc, OW], f32, name="Yf", tag="Yf")
                    nc.scalar.copy(out=Yf, in_=Yh)
                    nc.scalar.dma_start(
                        out=y_ext[s0 : s0 + 2, :, oh0 : oh0 + ohc, :], in_=Yf
                    )
    nc.compile()
    return nc


def _get_nc():
    if "nc" not in _cache:
        _cache["nc"] = _build()
    return _cache["nc"]


def run(x: np.ndarray, **spmd_kwargs):
    """Run the SPMD kernel; returns the BassKernelResults (for tracing)."""
    from concourse.bass_utils import run_bass_kernel_spmd

    nc = _get_nc()
    xs = np.ascontiguousarray(x, dtype=np.float32).reshape(B * C, D, H, W)
    in_maps = [
        {"x_shard": np.ascontiguousarray(xs[SLICES_PER_CORE * i : SLICES_PER_CORE * (i + 1)])}
        for i in range(N_CORES)
    ]
    return run_bass_kernel_spmd(nc, in_maps, list(range(N_CORES)), **spmd_kwargs)


def kernel(x: np.ndarray) -> np.ndarray:
    res = run(x)
    out = np.stack([res.results[i]["y_shard"] for i in range(N_CORES)])
    return out.reshape(B, C, OD, OH, OW)
